# revision 18
# baseline (speedup 1.0000x reference)
"""Trainium2 Bass kernel for the spike-train CV (coefficient of variation) loss.

Problem: for each (batch, neuron) sequence of T=2000 time steps, spikes are
positions where x > 0.  The loss is MSE between per-sequence CV of the
inter-spike intervals (ISIs, unbiased std / mean, penalty 10.0 when fewer
than 3 spikes) and a per-neuron target.

Algorithm (per sequence; one ACT pass + one DVE scan + cheap reductions):
  q(t)  = [x(t) <= 0]                    (ACT Sigmoid(-1e30*x) -> exact {0,1},
                                          accum -> sum_q, count = T - sum_q)
  a(t)  = q(t) * (a(t-1) + 1)            (DVE tensor_tensor_scan mult/add:
                                          a = age since last spike, 0 at spikes)
  A     = sum_t a(t)                     (in-place TS accumulate / ACT copy)
  last  = T-1 - a(T-1)
  first = #{t < PFX : a(t) == t+1}       (prefix of the leading ramp; exact
                                          unless a row has >= PFX leading
                                          non-spikes, prob ~2^-PFX)
From the window-counting identity P = T(T+1)/2 - A (P = sum of 1+last-spike
-time) the ISI square sum collapses to
  s1 = last - first
  s2 = 2T*last - last^2 - first^2 - 2*first + 2*A + T - T^2
then cv = std/mean with torch-style unbiased variance, penalty when count<3.

Sharding: batch dim (B=8) across the 8 cores, embarrassingly parallel; host
transposes each core's slab to (N, T) so time lies along the SBUF free axis
(the scan direction) and sums the 8 per-core partial squared-error sums.
"""

import numpy as np

import concourse.bass as bass
import concourse.tile as tile
from concourse import mybir
from concourse.bass_utils import run_bass_kernel_spmd

B, T, N = 8, 2000, 2048
P = 128                 # SBUF partitions
NB = N // P             # 16 neuron groups per core
PFX = 128               # leading-ramp prefix length for `first`
F32 = mybir.dt.float32
F16 = mybir.dt.float16
BF16 = mybir.dt.bfloat16
A = mybir.AluOpType
AF = mybir.ActivationFunctionType
AX = mybir.AxisListType

_CACHE = {}


def _build(a_act=13, act_lag=8, q_bufs=16, xt_bufs=12, a_bufs=6,
           final_split=True, dma_only=False, stage=9, repeats=1):
    nc = bass.Bass("TRN2", target_bir_lowering=False, debug=False, num_devices=B)

    xT = nc.dram_tensor("xT", [N, T], F32, kind="ExternalInput").ap()
    iota = nc.dram_tensor("iota", [P, PFX], F16, kind="ExternalInput").ap()
    tgt = nc.dram_tensor("tgt", [P, NB], F32, kind="ExternalInput").ap()
    out = nc.dram_tensor("out", [P, 1], F32, kind="ExternalOutput").ap()

    with tile.TileContext(nc) as tc:
        with (
            tc.tile_pool(name="const", bufs=1) as const_pool,
            tc.tile_pool(name="stats", bufs=1) as stats_pool,
            tc.tile_pool(name="xload", bufs=xt_bufs) as xload,
            tc.tile_pool(name="qpool", bufs=q_bufs) as qpool,
            tc.tile_pool(name="apool", bufs=a_bufs) as apool,
            tc.tile_pool(name="scr", bufs=2) as scr,
            tc.tile_pool(name="fin", bufs=1) as fin,
        ):
            iota_t = const_pool.tile([P, PFX], F16, tag="iota")
            nc.gpsimd.dma_start(iota_t[:], iota[:])
            tgt_t = const_pool.tile([P, NB], F32, tag="tgt")

            sumq = stats_pool.tile([P, NB], F32, tag="sumq")
            St = stats_pool.tile([P, NB], F32, tag="St")
            fst = stats_pool.tile([P, NB], F32, tag="fst")
            lastp = stats_pool.tile([P, NB], F32, tag="lastp")

            # ---- final per-neuron algebra on [P, NB] f32 tiles ----
            tiles = {}

            def ft(tag):
                if tag not in tiles:
                    tiles[tag] = fin.tile([P, NB], F32, name=tag, tag=tag)
                return tiles[tag]

            def emit_final(lo, hi):
                sl = slice(lo, hi)

                def ts(out_t, in_t, s1_, s2_, op0, op1=None):
                    if op1 is None:
                        nc.vector.tensor_scalar(
                            out_t[:, sl], in_t[:, sl], s1_, None, op0=op0
                        )
                    else:
                        nc.vector.tensor_scalar(
                            out_t[:, sl], in_t[:, sl], s1_, s2_, op0=op0, op1=op1
                        )
                    return out_t

                def tt(out_t, a_, b, op):
                    nc.vector.tensor_tensor(
                        out_t[:, sl], a_[:, sl], b[:, sl], op=op
                    )
                    return out_t

                def stt(out_t, a_, scal, b, op0, op1):
                    nc.vector.scalar_tensor_tensor(
                        out_t[:, sl], a_[:, sl], scal, b[:, sl], op0=op0, op1=op1
                    )
                    return out_t

                cnt = ts(ft("cnt"), sumq, -1.0, float(T), A.mult, A.add)
                last = ts(ft("last"), lastp, -1.0, float(T - 1), A.mult, A.add)
                s1 = tt(ft("s1"), last, fst, A.subtract)
                k = ts(ft("k"), cnt, -1.0, None, A.add)
                e1 = ts(ft("e1"), last, 2.0 * T, None, A.mult)
                bb = tt(ft("bb"), last, last, A.mult)
                aa = tt(ft("aa"), fst, fst, A.mult)
                g1 = tt(ft("g1"), e1, bb, A.subtract)
                g2 = tt(ft("g2"), g1, aa, A.subtract)
                g3 = ts(ft("g3"), fst, -2.0, float(T) - float(T) * T,
                        A.mult, A.add)
                g4 = tt(ft("g4"), g2, g3, A.add)
                s2t = stt(ft("s2t"), St, 2.0, g4, A.mult, A.add)

                maxk = ts(ft("maxk"), k, 1.0, None, A.max)
                invk = ft("invk")
                nc.vector.reciprocal(invk[:, sl], maxk[:, sl])
                mean = tt(ft("mean"), s1, invk, A.mult)
                km1 = ts(ft("km1"), k, -1.0, 1.0, A.add, A.max)
                invkm1 = ft("invkm1")
                nc.vector.reciprocal(invkm1[:, sl], km1[:, sl])

                km2 = tt(ft("km2"), s1, mean, A.mult)
                d = tt(ft("d"), s2t, km2, A.subtract)
                var = tt(ft("var"), d, invkm1, A.mult)
                varc = ts(ft("varc"), var, 0.0, None, A.max)
                std = ft("std")
                nc.scalar.activation(std[:, sl], varc[:, sl], AF.Sqrt)

                dm = ts(ft("dm"), mean, -1.0, None, A.add)
                t4 = stt(ft("t4"), mean, 0.0, dm, A.is_gt, A.mult)
                denom = ts(ft("denom"), t4, 1.0, None, A.add)
                invden = ft("invden")
                nc.vector.reciprocal(invden[:, sl], denom[:, sl])
                cv = tt(ft("cv"), std, invden, A.mult)

                cm = ts(ft("cm"), cv, -10.0, None, A.add)
                t5 = stt(ft("t5"), cnt, 3.0, cm, A.is_ge, A.mult)
                cvs = ts(ft("cvs"), t5, 10.0, None, A.add)

                diff = tt(ft("diff"), cvs, tgt_t, A.subtract)
                tt(ft("sq"), diff, diff, A.mult)

            nc.sync.dma_start(tgt_t[:], tgt[:])

            for rep in range(repeats):
                xts = []
                for nb in range(NB):
                    xt = xload.tile([P, T], F32, tag="xt")
                    nc.sync.dma_start(xt[:], xT[nb * P:(nb + 1) * P, :])
                    xts.append(xt)
                if dma_only:
                    for nb in range(NB):
                        nc.vector.tensor_copy(
                            lastp[:, nb:nb + 1], xts[nb][:, 0:1]
                        )
                    continue

                qs = [None] * NB

                def emit_q(nb):
                    q = qpool.tile([P, T], BF16, tag="q")
                    nc.scalar.activation(
                        q[:], xts[nb][:], AF.Sigmoid, scale=-1.0e30,
                        accum_out=sumq[:, nb:nb + 1],
                    )
                    qs[nb] = q

                pend_act = []

                def emit_a_act(nb, a_t):
                    nc.scalar.activation(
                        a_t[:], a_t[:], AF.Copy, accum_out=St[:, nb:nb + 1]
                    )

                if stage >= 2:
                    for nb in range(min(act_lag, NB)):
                        emit_q(nb)

                for nb in range(NB):
                    if stage < 2:
                        continue
                    if nb + act_lag < NB:
                        emit_q(nb + act_lag)
                    if pend_act:
                        emit_a_act(*pend_act.pop(0))
                    if stage < 3:
                        continue
                    a_t = apool.tile([P, T], BF16, tag="a")
                    nc.vector.tensor_tensor_scan(
                        a_t[:], qs[nb][:], qs[nb][:], 0.0,
                        op0=A.mult, op1=A.add,
                    )
                    nc.vector.tensor_copy(lastp[:, nb:nb + 1], a_t[:, T - 1:T])
                    if stage < 4:
                        continue
                    # first = count of leading-ramp hits in the prefix
                    sc = scr.tile([P, PFX], BF16, tag="sc")
                    nc.vector.scalar_tensor_tensor(
                        sc[:], a_t[:, 0:PFX], 1.0, iota_t[:],
                        op0=A.mult, op1=A.is_equal,
                        accum_out=fst[:, nb:nb + 1],
                    )
                    # A-pass: ACT for the first a_act groups, DVE for the rest
                    if nb < a_act:
                        pend_act.append((nb, a_t))
                    else:
                        nc.vector.tensor_scalar(
                            a_t[:], a_t[:], 0.0, None, op0=A.add, op1=A.add,
                            accum_out=St[:, nb:nb + 1],
                        )
                    if final_split and nb == NB // 2 - 1 and a_act <= NB // 2:
                        while pend_act:
                            emit_a_act(*pend_act.pop(0))
                        emit_final(0, NB // 2)

                if stage < 4:
                    continue

                while pend_act:
                    emit_a_act(*pend_act.pop(0))

                if final_split and a_act <= NB // 2:
                    emit_final(NB // 2, NB)
                else:
                    emit_final(0, NB)

                red = fin.tile([P, 1], F32, tag="red")
                nc.vector.tensor_reduce(
                    red[:], ft("sq")[:], axis=AX.X, op=A.add
                )
                # store on the ACT HWDGE ring so it never head-of-line
                # blocks the next rep's loads on the SP ring
                nc.scalar.dma_start(out[:], red[:])

    return nc


def _legalize_waits(nc):
    """Hoist excess sync-waits onto standalone EventSemaphore instructions.

    Hardware instruction encodings hold a single sync-wait (EventSemaphore
    holds two); the deployed tile scheduler sometimes attaches more, which
    walrus codegen rejects ("Too many sync wait commands").  Splitting the
    extra waits into preceding same-engine EventSemaphore ops is exactly
    equivalent: the engine stalls on the standalone waits first.
    """
    f = nc.m.functions[0]
    for blk in f.blocks:
        newlist = []
        for inst in blk.instructions:
            si = inst.sync_info
            tname = type(inst).__name__
            waits = list(si.on_wait) if si is not None else []
            cap = 2 if tname == "InstEventSemaphore" else 1
            if len(waits) <= cap:
                newlist.append(inst)
                continue
            for j, w in enumerate(waits[:-1]):
                es = mybir.InstEventSemaphore(name=f"{inst.name}-hw{j}")
                es.engine = inst.engine
                es.sync_info = mybir.SyncInfo(on_wait=[w], on_update=[])
                newlist.append(es)
            inst.sync_info = mybir.SyncInfo(
                on_wait=[waits[-1]], on_update=list(si.on_update)
            )
            newlist.append(inst)
        blk.instructions = newlist
    return nc


def _get_nc(**flags):
    key = tuple(sorted(flags.items()))
    if key not in _CACHE:
        nc = _build(**flags)
        _legalize_waits(nc)
        _CACHE[key] = nc
    return _CACHE[key]


def kernel(output_spikes, target_cv):
    x = np.asarray(output_spikes, dtype=np.float32)
    tgt = np.asarray(target_cv, dtype=np.float32)
    assert x.shape == (B, T, N), x.shape

    iota_np = np.broadcast_to(
        (np.arange(PFX, dtype=np.float32) + 1.0).astype(np.float16), (P, PFX)
    ).copy()
    tgt_np = np.ascontiguousarray(tgt.reshape(NB, P).T)  # [P, NB]

    in_maps = []
    for b in range(B):
        in_maps.append({
            "xT": np.ascontiguousarray(x[b].T),  # (N, T)
            "iota": iota_np,
            "tgt": tgt_np,
        })

    nc = _get_nc()
    res = run_bass_kernel_spmd(nc, in_maps, list(range(B)))

    total = np.float64(0.0)
    for b in range(B):
        total += np.asarray(res.results[b]["out"], dtype=np.float64).sum()
    loss = total / float(B * N)
    return np.float32(loss)


# revision 19
# speedup vs baseline: 1.2685x; 1.2685x over previous
"""Trainium2 Bass kernel for the spike-train CV (coefficient of variation) loss.

Problem: for each (batch, neuron) sequence of T=2000 time steps, spikes are
positions where x > 0.  The loss is MSE between per-sequence CV of the
inter-spike intervals (ISIs, unbiased std / mean, penalty 10.0 when fewer
than 3 spikes) and a per-neuron target.

Algorithm (per sequence; one ACT pass + one DVE scan + cheap reductions):
  q(t)  = [x(t) <= 0]                    (ACT Sigmoid(-1e30*x) -> exact {0,1},
                                          accum -> sum_q, count = T - sum_q)
  a(t)  = q(t) * (a(t-1) + 1)            (DVE tensor_tensor_scan mult/add:
                                          a = age since last spike, 0 at spikes)
  A     = sum_t a(t)                     (in-place TS accumulate / ACT copy)
  last  = T-1 - a(T-1)
  first = #{t < PFX : a(t) == t+1}       (prefix of the leading ramp; exact
                                          unless a row has >= PFX leading
                                          non-spikes, prob ~2^-PFX)
From the window-counting identity P = T(T+1)/2 - A (P = sum of 1+last-spike
-time) the ISI square sum collapses to
  s1 = last - first
  s2 = 2T*last - last^2 - first^2 - 2*first + 2*A + T - T^2
then cv = std/mean with torch-style unbiased variance, penalty when count<3.

Sharding: batch dim (B=8) across the 8 cores, embarrassingly parallel; host
transposes each core's slab to (N, T) so time lies along the SBUF free axis
(the scan direction) and sums the 8 per-core partial squared-error sums.
"""

import numpy as np

import concourse.bass as bass
import concourse.tile as tile
from concourse import mybir
from concourse.bass_utils import run_bass_kernel_spmd

B, T, N = 8, 2000, 2048
P = 128                 # SBUF partitions
NB = N // P             # 16 neuron groups per core
PFX = 128               # leading-ramp prefix length for `first`
F32 = mybir.dt.float32
F16 = mybir.dt.float16
BF16 = mybir.dt.bfloat16
A = mybir.AluOpType
AF = mybir.ActivationFunctionType
AX = mybir.AxisListType

_CACHE = {}


def _build(a_act=13, act_lag=8, q_bufs=16, xt_bufs=12, a_bufs=6,
           final_split=True, dma_only=False, stage=9, repeats=1,
           a_act_tail=False):
    nc = bass.Bass("TRN2", target_bir_lowering=False, debug=False, num_devices=B)

    xT = nc.dram_tensor("xT", [N, T], F32, kind="ExternalInput").ap()
    iota = nc.dram_tensor("iota", [P, PFX], F16, kind="ExternalInput").ap()
    tgt = nc.dram_tensor("tgt", [P, NB], F32, kind="ExternalInput").ap()
    out = nc.dram_tensor("out", [P, 1], F32, kind="ExternalOutput").ap()

    with tile.TileContext(nc) as tc:
        with (
            tc.tile_pool(name="const", bufs=1) as const_pool,
            tc.tile_pool(name="stats", bufs=1) as stats_pool,
            tc.tile_pool(name="xload", bufs=xt_bufs) as xload,
            tc.tile_pool(name="qpool", bufs=q_bufs) as qpool,
            tc.tile_pool(name="apool", bufs=a_bufs) as apool,
            tc.tile_pool(name="scr", bufs=2) as scr,
            tc.tile_pool(name="fin", bufs=1) as fin,
        ):
            iota_t = const_pool.tile([P, PFX], F16, tag="iota")
            nc.gpsimd.dma_start(iota_t[:], iota[:])
            tgt_t = const_pool.tile([P, NB], F32, tag="tgt")

            sumq = stats_pool.tile([P, NB], F32, tag="sumq")
            St = stats_pool.tile([P, NB], F32, tag="St")
            fst = stats_pool.tile([P, NB], F32, tag="fst")
            lastp = stats_pool.tile([P, NB], F32, tag="lastp")

            # ---- final per-neuron algebra on [P, NB] f32 tiles ----
            tiles = {}

            def ft(tag):
                if tag not in tiles:
                    tiles[tag] = fin.tile([P, NB], F32, name=tag, tag=tag)
                return tiles[tag]

            def emit_final(lo, hi):
                sl = slice(lo, hi)

                def ts(out_t, in_t, s1_, s2_, op0, op1=None):
                    if op1 is None:
                        nc.vector.tensor_scalar(
                            out_t[:, sl], in_t[:, sl], s1_, None, op0=op0
                        )
                    else:
                        nc.vector.tensor_scalar(
                            out_t[:, sl], in_t[:, sl], s1_, s2_, op0=op0, op1=op1
                        )
                    return out_t

                def tt(out_t, a_, b, op):
                    nc.vector.tensor_tensor(
                        out_t[:, sl], a_[:, sl], b[:, sl], op=op
                    )
                    return out_t

                def stt(out_t, a_, scal, b, op0, op1):
                    nc.vector.scalar_tensor_tensor(
                        out_t[:, sl], a_[:, sl], scal, b[:, sl], op0=op0, op1=op1
                    )
                    return out_t

                cnt = ts(ft("cnt"), sumq, -1.0, float(T), A.mult, A.add)
                last = ts(ft("last"), lastp, -1.0, float(T - 1), A.mult, A.add)
                s1 = tt(ft("s1"), last, fst, A.subtract)
                k = ts(ft("k"), cnt, -1.0, None, A.add)
                e1 = ts(ft("e1"), last, 2.0 * T, None, A.mult)
                bb = tt(ft("bb"), last, last, A.mult)
                aa = tt(ft("aa"), fst, fst, A.mult)
                g1 = tt(ft("g1"), e1, bb, A.subtract)
                g2 = tt(ft("g2"), g1, aa, A.subtract)
                g3 = ts(ft("g3"), fst, -2.0, float(T) - float(T) * T,
                        A.mult, A.add)
                g4 = tt(ft("g4"), g2, g3, A.add)
                s2t = stt(ft("s2t"), St, 2.0, g4, A.mult, A.add)

                maxk = ts(ft("maxk"), k, 1.0, None, A.max)
                invk = ft("invk")
                nc.vector.reciprocal(invk[:, sl], maxk[:, sl])
                mean = tt(ft("mean"), s1, invk, A.mult)
                km1 = ts(ft("km1"), k, -1.0, 1.0, A.add, A.max)
                invkm1 = ft("invkm1")
                nc.vector.reciprocal(invkm1[:, sl], km1[:, sl])

                km2 = tt(ft("km2"), s1, mean, A.mult)
                d = tt(ft("d"), s2t, km2, A.subtract)
                var = tt(ft("var"), d, invkm1, A.mult)
                varc = ts(ft("varc"), var, 0.0, None, A.max)
                std = ft("std")
                nc.scalar.activation(std[:, sl], varc[:, sl], AF.Sqrt)

                dm = ts(ft("dm"), mean, -1.0, None, A.add)
                t4 = stt(ft("t4"), mean, 0.0, dm, A.is_gt, A.mult)
                denom = ts(ft("denom"), t4, 1.0, None, A.add)
                invden = ft("invden")
                nc.vector.reciprocal(invden[:, sl], denom[:, sl])
                cv = tt(ft("cv"), std, invden, A.mult)

                cm = ts(ft("cm"), cv, -10.0, None, A.add)
                t5 = stt(ft("t5"), cnt, 3.0, cm, A.is_ge, A.mult)
                cvs = ts(ft("cvs"), t5, 10.0, None, A.add)

                diff = tt(ft("diff"), cvs, tgt_t, A.subtract)
                tt(ft("sq"), diff, diff, A.mult)

            nc.sync.dma_start(tgt_t[:], tgt[:])

            for rep in range(repeats):
                xts = []
                for nb in range(NB):
                    xt = xload.tile([P, T], F32, tag="xt")
                    nc.sync.dma_start(xt[:], xT[nb * P:(nb + 1) * P, :])
                    xts.append(xt)
                if dma_only:
                    for nb in range(NB):
                        nc.vector.tensor_copy(
                            lastp[:, nb:nb + 1], xts[nb][:, 0:1]
                        )
                    continue

                qs = [None] * NB

                def emit_q(nb):
                    q = qpool.tile([P, T], BF16, tag="q")
                    nc.scalar.activation(
                        q[:], xts[nb][:], AF.Sigmoid, scale=-1.0e30,
                        accum_out=sumq[:, nb:nb + 1],
                    )
                    qs[nb] = q

                pend_act = []

                def emit_a_act(nb, a_t):
                    nc.scalar.activation(
                        a_t[:], a_t[:], AF.Copy, accum_out=St[:, nb:nb + 1]
                    )

                if stage >= 2:
                    for nb in range(min(act_lag, NB)):
                        emit_q(nb)

                for nb in range(NB):
                    if stage < 2:
                        continue
                    if nb + act_lag < NB:
                        emit_q(nb + act_lag)
                    if pend_act:
                        emit_a_act(*pend_act.pop(0))
                    if stage < 3:
                        continue
                    a_t = apool.tile([P, T], BF16, tag="a")
                    nc.vector.tensor_tensor_scan(
                        a_t[:], qs[nb][:], qs[nb][:], 0.0,
                        op0=A.mult, op1=A.add,
                    )
                    nc.vector.tensor_copy(lastp[:, nb:nb + 1], a_t[:, T - 1:T])
                    if stage < 4:
                        continue
                    # first = count of leading-ramp hits in the prefix
                    sc = scr.tile([P, PFX], BF16, tag="sc")
                    nc.vector.scalar_tensor_tensor(
                        sc[:], a_t[:, 0:PFX], 1.0, iota_t[:],
                        op0=A.mult, op1=A.is_equal,
                        accum_out=fst[:, nb:nb + 1],
                    )
                    # A-pass: ACT for the first a_act groups (or the last,
                    # with a_act_tail), DVE for the rest
                    if (nb >= NB - a_act) if a_act_tail else (nb < a_act):
                        pend_act.append((nb, a_t))
                    else:
                        nc.vector.tensor_scalar(
                            a_t[:], a_t[:], 0.0, None, op0=A.add, op1=A.add,
                            accum_out=St[:, nb:nb + 1],
                        )
                    if final_split and nb == NB // 2 - 1 and (
                            (a_act <= NB // 2) if not a_act_tail
                            else (NB - a_act >= NB // 2)):
                        while pend_act:
                            emit_a_act(*pend_act.pop(0))
                        emit_final(0, NB // 2)

                if stage < 4:
                    continue

                while pend_act:
                    emit_a_act(*pend_act.pop(0))

                if final_split and ((a_act <= NB // 2) if not a_act_tail
                                    else (NB - a_act >= NB // 2)):
                    emit_final(NB // 2, NB)
                else:
                    emit_final(0, NB)

                red = fin.tile([P, 1], F32, tag="red")
                nc.vector.tensor_reduce(
                    red[:], ft("sq")[:], axis=AX.X, op=A.add
                )
                # store on the ACT HWDGE ring so it never head-of-line
                # blocks the next rep's loads on the SP ring
                nc.scalar.dma_start(out[:], red[:])

    return nc


def _legalize_waits(nc):
    """Hoist excess sync-waits onto standalone EventSemaphore instructions.

    Hardware instruction encodings hold a single sync-wait (EventSemaphore
    holds two); the deployed tile scheduler sometimes attaches more, which
    walrus codegen rejects ("Too many sync wait commands").  Splitting the
    extra waits into preceding same-engine EventSemaphore ops is exactly
    equivalent: the engine stalls on the standalone waits first.
    """
    f = nc.m.functions[0]
    for blk in f.blocks:
        newlist = []
        for inst in blk.instructions:
            si = inst.sync_info
            tname = type(inst).__name__
            waits = list(si.on_wait) if si is not None else []
            cap = 2 if tname == "InstEventSemaphore" else 1
            if len(waits) <= cap:
                newlist.append(inst)
                continue
            for j, w in enumerate(waits[:-1]):
                es = mybir.InstEventSemaphore(name=f"{inst.name}-hw{j}")
                es.engine = inst.engine
                es.sync_info = mybir.SyncInfo(on_wait=[w], on_update=[])
                newlist.append(es)
            inst.sync_info = mybir.SyncInfo(
                on_wait=[waits[-1]], on_update=list(si.on_update)
            )
            newlist.append(inst)
        blk.instructions = newlist
    return nc


def _get_nc(**flags):
    key = tuple(sorted(flags.items()))
    if key not in _CACHE:
        nc = _build(**flags)
        _legalize_waits(nc)
        _CACHE[key] = nc
    return _CACHE[key]


def kernel(output_spikes, target_cv):
    x = np.asarray(output_spikes, dtype=np.float32)
    tgt = np.asarray(target_cv, dtype=np.float32)
    assert x.shape == (B, T, N), x.shape

    iota_np = np.broadcast_to(
        (np.arange(PFX, dtype=np.float32) + 1.0).astype(np.float16), (P, PFX)
    ).copy()
    tgt_np = np.ascontiguousarray(tgt.reshape(NB, P).T)  # [P, NB]

    in_maps = []
    for b in range(B):
        in_maps.append({
            "xT": np.ascontiguousarray(x[b].T),  # (N, T)
            "iota": iota_np,
            "tgt": tgt_np,
        })

    nc = _get_nc()
    res = run_bass_kernel_spmd(nc, in_maps, list(range(B)))

    total = np.float64(0.0)
    for b in range(B):
        total += np.asarray(res.results[b]["out"], dtype=np.float64).sum()
    loss = total / float(B * N)
    return np.float32(loss)
